# revision 14
# baseline (speedup 1.0000x reference)
"""BotRGCN Trainium2 kernel: feature transform + 2 RGCN layers + classifier.

Sharding: nodes split across 8 cores by id (12500/core, padded to 12544).
Edges partitioned by destination shard; per (relation, dst-window, src-bank)
groups padded to a block structure uniform across cores so a single SPMD
program serves all 8 cores. Source features exchanged via bf16 AllGather of
the per-layer node-feature table; gathers via int16 dma_gather per src bank,
spread over 4 SWDGE queues so descriptor generation overlaps ring drain.

Aggregation is a pure sum via one-hot scatter matmuls (one-hot tiles built in
batched DVE is_equal ops); the per-(rel, dst) mean reciprocal is applied
afterwards via a rank-1 broadcast matmul + elementwise multiply.
"""

import sys

sys.path.insert(0, "/opt/trn_rl_repo")

from contextlib import ExitStack

import numpy as np
import ml_dtypes

import concourse.bass as bass
import concourse.bacc as bacc
import concourse.mybir as mybir
import concourse.tile as tile
from concourse.masks import make_identity
from concourse.bass_utils import run_bass_kernel_spmd

BF16 = mybir.dt.bfloat16
F32 = mybir.dt.float32
I16 = mybir.dt.int16

P = 128

# full-problem config (test.py overrides for mini runs)
CFG = dict(
    N=100000,        # nodes
    NC=8,            # cores
    R=2,             # relations
    H=128,
    DES=768, TWEET=768, NUMP=6, CATP=11,
    WIN=256,         # dst window (PSUM free dim)
    NBLK_CH=16,      # gather-chunk size in 128-edge blocks
    SCH=8,           # one-hot build chunk size in blocks
    BANKROWS=25088,  # gather-table bank rows (< 2^15)
    NTF=512,         # feature-stage node tile
)


def _derived(cfg):
    d = dict(cfg)
    d["SH"] = cfg["N"] // cfg["NC"]
    d["SHP"] = ((d["SH"] + P - 1) // P) * P
    d["NW"] = d["SHP"] // cfg["WIN"]
    assert d["SHP"] % cfg["WIN"] == 0
    d["TROWS"] = cfg["NC"] * d["SHP"]           # padded table rows
    # bank-aligned table chunks: per-core rows per chunk (last one short)
    full = (d["SHP"] + 4 - 1) // 4
    full = ((full + P - 1) // P) * P
    chg = []
    left = d["SHP"]
    while left > 0:
        take = min(full, left)
        chg.append(take)
        left -= take
    d["CHG"] = chg
    assert all(c * cfg["NC"] < 2 ** 15 for c in chg)
    d["BANKS"] = len(chg)
    d["TBLK"] = d["SHP"] // P                   # 128-row blobs per core
    # x feature layout: [des | tweet | num(pad to 128) | cat(pad to 128)]
    d["KDES"] = cfg["DES"] // P
    d["KTWEET"] = cfg["TWEET"] // P
    d["KX"] = d["KDES"] + d["KTWEET"] + 2
    d["XROWS"] = d["KX"] * P
    return d


# ---------------------------------------------------------------------------
# host-side graph planning
# ---------------------------------------------------------------------------

class Plan:
    pass


def build_plan(edge_index, edge_type, cfg):
    """Group edges per core by (rel, dst-window, src-bank); pad each group to a
    whole number of 128-edge blocks, uniform across cores. Returns per-core
    gather-index / meta arrays plus the uniform block structure."""
    d = cfg
    NC, SH, SHP, WIN, NW = d["NC"], d["SH"], d["SHP"], d["WIN"], d["NW"]
    BANKS, BR, NBLK_CH = d["BANKS"], d["BANKROWS"], d["NBLK_CH"]
    R = d["R"]
    N = d["N"]
    TBLK = d["TBLK"]

    src = np.asarray(edge_index[0], dtype=np.int64)
    dst = np.asarray(edge_index[1], dtype=np.int64)
    et = np.asarray(edge_type, dtype=np.int64)

    core = dst // SH
    dl = dst - core * SH
    # table layout: 4 bank-aligned chunks; within a chunk, rows are
    # [src-core][node-order]. CHG rows per core per chunk.
    CHG = d["CHG"]
    gof = np.concatenate([[0], np.cumsum(CHG)])       # per-core chunk offsets
    sl = src - (src // SH) * SH
    g = np.minimum(sl // CHG[0], len(CHG) - 1)
    bank = g
    bidx = (gof[g] * 0 + (src // SH) * np.asarray(CHG)[g]
            + (sl - gof[g])).astype(np.int16)
    win = dl // WIN
    dw = (dl - win * WIN).astype(np.float32)

    # per-(rel, node) in-degree -> per-core recip table [R, SHP]
    cnt = np.bincount(et * N + dst, minlength=R * N).reshape(R, N)
    recip_full = (1.0 / np.maximum(cnt, 1.0)).astype(np.float32)   # [R, N]
    recip = np.zeros((NC, R, SHP), np.float32)
    for c in range(NC):
        recip[c, :, :SH] = recip_full[:, c * SH:(c + 1) * SH]

    # group = (bank, win) with edges rel-sorted inside; uniform block counts
    # = max over cores. Slot space is bank-major so each bank is ONE gather
    # stream consumed sequentially by the (w, r) window loop. A block may mix
    # relations near the per-group rel boundary; each (block, rel) pair any
    # core needs gets its own one-hot meta entry (non-target edges stay -1).
    NG = BANKS * NW
    gid = bank * NW + win
    counts = np.bincount(core * NG + gid, minlength=NC * NG).reshape(NC, NG)
    bpg_bw = (counts.max(axis=0) + P - 1) // P         # blocks per group
    bpg_bw = bpg_bw.reshape(BANKS, NW).copy()
    # pad each bank stream to a whole number of chunks (extra blocks to the
    # stream's last group)
    for b in range(BANKS):
        tot = int(bpg_bw[b].sum())
        pad = (-tot) % NBLK_CH
        if tot == 0 and pad == 0:
            pad = NBLK_CH
        bpg_bw[b, NW - 1] += pad

    slots_per_group = (bpg_bw.reshape(-1) * P)
    slot_base = np.zeros(NG + 1, np.int64)
    np.cumsum(slots_per_group, out=slot_base[1:])
    TOTSLOT = int(slot_base[-1])
    TOTBLK = TOTSLOT // P

    # stream bookkeeping: stream = bank; block base per (b, w) group
    group_blk_base = np.zeros((BANKS, NW), np.int64)
    base = 0
    stream_blk_base = np.zeros(BANKS, np.int64)
    for b in range(BANKS):
        stream_blk_base[b] = base
        for w in range(NW):
            group_blk_base[b, w] = base
            base += int(bpg_bw[b, w])
    assert base == TOTBLK

    # per-(core, b, w): r0 edge count -> which blocks each rel touches.
    # needs[r][b][w] = sorted list of block indices k (within the group) that
    # hold rel-r edges on ANY core (always also emit block 0 of rel r's side
    # if group nonempty on any core? no: derive purely from counts).
    cnt0 = np.bincount((core * NG + gid)[et == 0], minlength=NC * NG)
    cnt0 = cnt0.reshape(NC, NG).reshape(NC, BANKS, NW)
    cnt01 = counts.reshape(NC, BANKS, NW)
    needs = [[[None] * NW for _ in range(BANKS)] for _ in range(R)]
    for b in range(BANKS):
        for w in range(NW):
            nb = int(bpg_bw[b, w])
            n0 = cnt0[:, b, w]
            n01 = cnt01[:, b, w]
            k0 = set()
            k1 = set()
            for c in range(NC):
                if n0[c] > 0:
                    k0.update(range(0, (int(n0[c]) + P - 1) // P))
                if n01[c] > int(n0[c]):
                    k1.update(range(int(n0[c]) // P, (int(n01[c]) + P - 1) // P))
            needs[0][b][w] = sorted(k0)
            needs[1][b][w] = sorted(k1)

    # st order: (w, r, b, needed-k) — the one-hot consumption order
    st_entry = {}
    sbase = 0
    st_group_base = np.zeros((NW, R, BANKS), np.int64)
    for w in range(NW):
        for r in range(R):
            for b in range(BANKS):
                st_group_base[w, r, b] = sbase
                for k in needs[r][b][w]:
                    st_entry[(r, b, w, k)] = sbase
                    sbase += 1
    ST_TOT = sbase

    # place each edge into its group's slot range (per core), rel-sorted
    okey = (core * NG + gid) * R + et
    order = np.argsort(okey, kind="stable")
    gkey = (core * NG + gid)[order]
    first_of = np.r_[True, gkey[1:] != gkey[:-1]]
    idx_in_run = np.arange(len(gkey)) - np.maximum.accumulate(
        np.where(first_of, np.arange(len(gkey)), 0)
    )
    slot = slot_base[gkey % NG] + idx_in_run

    idx16 = np.zeros((NC, 8 * 16, TOTSLOT // 16), np.int16)
    meta = np.full((NC, P, max(ST_TOT, 1)), -1.0, np.float32)
    ecore = core[order]
    col = slot // 16
    prow = (slot % 16).astype(np.int64)
    for g2 in range(8):
        idx16[ecore, 16 * g2 + prow, col] = bidx[order]
    # meta entry per (rel, block): position of each edge within its block
    eb = np.asarray(bank)[order]
    ew = win[order]
    er = et[order]
    ek = (slot - slot_base[gkey % NG]) // P
    es = np.fromiter(
        (st_entry[(int(er[i]), int(eb[i]), int(ew[i]), int(ek[i]))]
         for i in range(len(order))), np.int64, len(order))
    meta[ecore, slot % P, es] = dw[order]

    pl = Plan()
    pl.idx16 = idx16.reshape(NC, P, TOTSLOT // 16)
    pl.meta = meta.astype(ml_dtypes.bfloat16)
    pl.recip = recip.astype(ml_dtypes.bfloat16)
    pl.TOTBLK = TOTBLK
    pl.ST_TOT = max(ST_TOT, 1)
    pl.needs = needs
    pl.group_blk_base = group_blk_base
    pl.stream_blk_base = stream_blk_base
    pl.st_group_base = st_group_base
    return pl


def prep_x(x, cfg):
    """Per-core transposed bf16 feature blocks [XROWS, SHP]."""
    d = cfg
    NC, SH, SHP = d["NC"], d["SH"], d["SHP"]
    NUMP, TWEET, CATP, DES = d["NUMP"], d["TWEET"], d["CATP"], d["DES"]
    KD, KT = d["KDES"], d["KTWEET"]
    out = np.zeros((NC, d["XROWS"], SHP), ml_dtypes.bfloat16)
    for c in range(NC):
        xs = x[c * SH:(c + 1) * SH]
        xT = np.zeros((d["XROWS"], SHP), np.float32)
        xT[:DES, :SH] = xs[:, NUMP + TWEET + CATP:].T
        xT[DES:DES + TWEET, :SH] = xs[:, NUMP:NUMP + TWEET].T
        xT[(KD + KT) * P:(KD + KT) * P + NUMP, :SH] = xs[:, :NUMP].T
        xT[(KD + KT + 1) * P:(KD + KT + 1) * P + CATP, :SH] = \
            xs[:, NUMP + TWEET:NUMP + TWEET + CATP].T
        out[c] = xT.astype(ml_dtypes.bfloat16)
    return out


def prep_weights(inp, cfg):
    """bf16 weight blocks + packed fp32 biases."""
    bf = lambda a: np.asarray(a, np.float32).astype(ml_dtypes.bfloat16)
    d = cfg
    wnum = np.zeros((P, d["H"]), np.float32)
    wnum[:d["NUMP"]] = inp["W_num"]
    wcat = np.zeros((P, d["H"]), np.float32)
    wcat[:d["CATP"]] = inp["W_cat"]
    w = {
        "wdes": bf(inp["W_des"]), "wtweet": bf(inp["W_tweet"]),
        "wnum": bf(wnum), "wcat": bf(wcat), "win": bf(inp["W_in"]),
        "root1": bf(inp["root1"]), "rel10": bf(inp["rel1"][0]),
        "rel11": bf(inp["rel1"][1]),
        "root2": bf(inp["root2"]), "rel20": bf(inp["rel2"][0]),
        "rel21": bf(inp["rel2"][1]), "wcls": bf(inp["W_cls"]),
    }
    biases = np.stack(
        [inp["b_des"], inp["b_tweet"], inp["b_num"], inp["b_cat"],
         inp["b_in"], inp["prelu_a"], inp["bias1"], inp["bias2"],
         inp["b_cls"]], axis=1).astype(np.float32)   # [128, 9]
    w["biases"] = biases
    return w


# ---------------------------------------------------------------------------
# bass program
# ---------------------------------------------------------------------------

def build_bass(cfg, pl):
    d = cfg
    NC, SHP, WIN, NW, NTF = d["NC"], d["SHP"], d["WIN"], d["NW"], d["NTF"]
    BANKS, BR, NBLK_CH = d["BANKS"], d["BANKROWS"], d["NBLK_CH"]
    R, H = d["R"], d["H"]
    KD, KT, KX = d["KDES"], d["KTWEET"], d["KX"]
    TBLK = d["TBLK"]
    TROWS = d["TROWS"]
    SCH = d["SCH"]
    CHS = NBLK_CH * P      # idx slots per chunk

    nc = bacc.Bacc(None, target_bir_lowering=False, debug=False,
                   num_devices=NC, num_swdge_queues=4,
                   dynamic_dma_scratch_size=32768)

    # ---- I/O ----
    xT = nc.dram_tensor("xT", [d["XROWS"], SHP], BF16, kind="ExternalInput")
    idxt = nc.dram_tensor("idxt", [P, pl.TOTBLK * P // 16], I16, kind="ExternalInput")
    metat = nc.dram_tensor("metat", [P, pl.ST_TOT], BF16, kind="ExternalInput")
    recipt = nc.dram_tensor("recipt", [R, SHP], BF16, kind="ExternalInput")
    wts = {}
    for nm, shp in [("wdes", [d["DES"], H]), ("wtweet", [d["TWEET"], H]),
                    ("wnum", [P, H]), ("wcat", [P, H]), ("win", [4 * P, H]),
                    ("root1", [H, H]), ("rel10", [H, H]), ("rel11", [H, H]),
                    ("root2", [H, H]), ("rel20", [H, H]), ("rel21", [H, H]),
                    ("wcls", [H, H])]:
        wts[nm] = nc.dram_tensor(nm, shp, BF16, kind="ExternalInput")
    biases = nc.dram_tensor("biases", [P, 9], F32, kind="ExternalInput")
    outT = nc.dram_tensor("outT", [P, SHP], F32, kind="ExternalOutput")

    # ---- collective tables (bank-aligned chunks) ----
    CHG = d["CHG"]
    cc_in = [[nc.dram_tensor(f"cc{i}_in{g}", [CHG[g], H], BF16,
                             kind="Internal")
              for g in range(len(CHG))] for i in (1, 2)]
    cc_out = [[nc.dram_tensor(f"cc{i}_out{g}", [NC * CHG[g], H], BF16,
                              kind="Internal", addr_space="Shared")
               for g in range(len(CHG))] for i in (1, 2)]

    rg = [list(range(NC))]

    with tile.TileContext(nc) as tc:
        with (
            tc.tile_pool(name="const", bufs=1) as cpool,
            tc.tile_pool(name="resident", bufs=1) as rpool,
            ExitStack() as mstack,
        ):
            # ---- constants ----
            ident = cpool.tile([P, P], BF16)
            make_identity(nc, ident[:])
            # replicated iota: value at (nb, w) = w
            iota = cpool.tile([P, SCH, WIN], BF16)
            nc.gpsimd.iota(iota[:], pattern=[[0, SCH], [1, WIN]], base=0,
                           channel_multiplier=0,
                           allow_small_or_imprecise_dtypes=True)
            ones = cpool.tile([1, P], BF16)
            nc.vector.memset(ones[:], 1.0)
            bias_t = cpool.tile([P, 9], F32)
            nc.sync.dma_start(out=bias_t[:], in_=biases[:])
            recip_sb = cpool.tile([1, R * SHP], BF16)
            nc.sync.dma_start(
                out=recip_sb[:], in_=recipt.rearrange("r n -> (r n)")
                .unsqueeze(0))

            wt = {}
            for nm, kb in [("wdes", KD), ("wtweet", KT), ("wnum", 1),
                           ("wcat", 1), ("win", 4), ("root1", 1),
                           ("rel10", 1), ("rel11", 1), ("root2", 1),
                           ("rel20", 1), ("rel21", 1), ("wcls", 1)]:
                t = cpool.tile([P, kb, H], BF16, tag=f"w_{nm}", name=f"w_{nm}")
                nc.sync.dma_start(
                    out=t[:], in_=wts[nm].rearrange("(k p) h -> p k h", p=P))
                wt[nm] = t

            # resident activations (transposed, [H, SHP] bf16)
            hT = [rpool.tile([P, SHP], BF16, tag="ht", name=f"hT{i}", bufs=2)
                  for i in range(3)]

            # =============== feature transform ===============
            fstack = ExitStack()
            fpool = fstack.enter_context(tc.tile_pool(name="featsb", bufs=2))
            fpp = fstack.enter_context(
                tc.tile_pool(name="featps", bufs=2, space="PSUM"))
            ntiles = (SHP + NTF - 1) // NTF
            for t in range(ntiles):
                n0 = t * NTF
                n1 = min(SHP, n0 + NTF)
                nn = n1 - n0
                xt = fpool.tile([P, KX, NTF], BF16, tag="xt", name="xt")
                nc.sync.dma_start(
                    out=xt[:, :, :nn],
                    in_=xT.rearrange("(k p) n -> p k n", p=P)[:, :, n0:n1])

                zb = []
                for bi, (wnm, ks, kn) in enumerate([
                        ("wdes", 0, KD), ("wtweet", KD, KT),
                        ("wnum", KD + KT, 1), ("wcat", KD + KT + 1, 1)]):
                    pz = fpp.tile([P, NTF], F32, tag=f"pz{bi}", name=f"pz{bi}", space="PSUM", bufs=1)
                    for k in range(kn):
                        nc.tensor.matmul(
                            out=pz[:, :nn], lhsT=wt[wnm][:, k, :],
                            rhs=xt[:, ks + k, :nn],
                            start=(k == 0), stop=(k == kn - 1))
                    v = fpool.tile([P, NTF], BF16, tag=f"v{bi}", name=f"v{bi}")
                    nc.scalar.activation(
                        out=v[:, :nn], in_=pz[:, :nn],
                        func=mybir.ActivationFunctionType.Identity,
                        bias=bias_t[:, bi:bi + 1])
                    z = fpool.tile([P, NTF], BF16, tag=f"z{bi}", name=f"z{bi}")
                    nc.vector.scalar_tensor_tensor(
                        out=z[:, :nn], in0=v[:, :nn], scalar=0.01,
                        in1=v[:, :nn], op0=mybir.AluOpType.mult,
                        op1=mybir.AluOpType.max)
                    zb.append(z)

                ph = fpp.tile([P, NTF], F32, tag="ph", name="ph", space="PSUM")
                for k in range(4):
                    nc.tensor.matmul(out=ph[:, :nn], lhsT=wt["win"][:, k, :],
                                     rhs=zb[k][:, :nn],
                                     start=(k == 0), stop=(k == 3))
                vh = fpool.tile([P, NTF], F32, tag="vh", name="vh")
                nc.scalar.activation(
                    out=vh[:, :nn], in_=ph[:, :nn],
                    func=mybir.ActivationFunctionType.Identity,
                    bias=bias_t[:, 4:5])
                nc.vector.scalar_tensor_tensor(
                    out=hT[0][:, n0:n1], in0=vh[:, :nn],
                    scalar=bias_t[:, 5:6], in1=vh[:, :nn],
                    op0=mybir.AluOpType.mult, op1=mybir.AluOpType.max)

            fstack.close()
            wpool = mstack.enter_context(tc.tile_pool(name="work", bufs=3))
            ppool = mstack.enter_context(
                tc.tile_pool(name="psum", bufs=2, space="PSUM"))

            # resident meta (dw per st-ordered block), reloaded per layer
            meta_sb = rpool.tile([P, pl.ST_TOT], BF16, tag="meta",
                                 name="meta", bufs=1)

            # =============== per-layer helpers ===============
            chunk_blk0 = [0]
            for g in range(len(CHG)):
                chunk_blk0.append(chunk_blk0[-1] + CHG[g] // P)

            def emit_table_blk(src_hT, li, blk):
                g = 0
                while blk >= chunk_blk0[g + 1]:
                    g += 1
                lb = blk - chunk_blk0[g]
                tp = ppool.tile([P, P], BF16, tag="tp", name="tp", space="PSUM", bufs=2)
                nc.tensor.transpose(
                    out=tp[:], in_=src_hT[:, blk * P:(blk + 1) * P],
                    identity=ident[:])
                rowt = wpool.tile([P, P], BF16, tag="rowt", name="rowt",
                                  bufs=3)
                nc.scalar.copy(out=rowt[:], in_=tp[:])
                nc.sync.dma_start(out=cc_in[li][g][lb * P:(lb + 1) * P, :],
                                  in_=rowt[:])
                if blk + 1 == chunk_blk0[g + 1]:
                    nc.gpsimd.collective_compute(
                        "AllGather", mybir.AluOpType.bypass,
                        ins=[cc_in[li][g][:]], outs=[cc_out[li][g][:]],
                        replica_groups=rg)

            def emit_table(src_hT, li):
                for blk in range(TBLK):
                    emit_table_blk(src_hT, li, blk)

            def emit_layer(li, h_in, h_out, tables, rootw, relw, bias_col,
                           emit_next=None):
                # per-stream gather state
                cur = {}
                cur_st = {}

                def ensure_chunk(b, blkloc):
                    ch = blkloc // NBLK_CH
                    if cur.get(b, (-1,))[0] == ch:
                        return cur[b]
                    gblk0 = int(pl.stream_blk_base[b]) + ch * NBLK_CH
                    it = wpool.tile([P, CHS // 16], I16, tag=f"idx{b}", name=f"idx{b}", bufs=3)
                    nc.sync.dma_start(
                        out=it[:],
                        in_=idxt[:, gblk0 * P // 16:(gblk0 + NBLK_CH) * P // 16])
                    gt = wpool.tile([P, NBLK_CH, P], BF16, tag=f"gt{b}", name=f"gt{b}", bufs=3)
                    nc.gpsimd.dma_gather(
                        out_ap=gt[:],
                        in_ap=tables[b][:],
                        idxs_ap=it[:], num_idxs=CHS, num_idxs_reg=CHS,
                        elem_size=H, single_packet=False, queue_num=b % 4)
                    cur[b] = (ch, gt)
                    return cur[b]

                def ensure_st(stblk):
                    ch = stblk // SCH
                    if cur_st.get("c", -1) == ch:
                        return cur_st["t"]
                    stt = wpool.tile([P, SCH, WIN], BF16, tag="onehot",
                                     name="onehot", bufs=3)
                    m0 = ch * SCH
                    nc.vector.tensor_tensor(
                        out=stt[:],
                        in0=iota[:],
                        in1=meta_sb[:, m0:m0 + SCH].unsqueeze(2)
                            .to_broadcast([P, SCH, WIN]),
                        op=mybir.AluOpType.is_equal)
                    cur_st["c"] = ch
                    cur_st["t"] = stt
                    return stt

                for w in range(NW):
                    ws = slice(w * WIN, (w + 1) * WIN)
                    agg = []
                    for r in range(R):
                        pa = ppool.tile([P, WIN], F32, tag=f"agg{r}", name=f"agg{r}",
                                        space="PSUM", bufs=1)
                        nblk_w = sum(len(pl.needs[r][b][w])
                                     for b in range(BANKS))
                        j = 0
                        stbase = int(pl.st_group_base[w, r, 0])
                        for b in range(BANKS):
                            base = int(pl.group_blk_base[b, w]
                                       - pl.stream_blk_base[b])
                            for k in pl.needs[r][b][w]:
                                blkloc = base + k
                                ch, gt = ensure_chunk(b, blkloc)
                                pos = blkloc - ch * NBLK_CH
                                stblk = stbase + j
                                stt = ensure_st(stblk)
                                spos = stblk - (stblk // SCH) * SCH
                                nc.tensor.matmul(
                                    out=pa[:], lhsT=gt[:, pos, :],
                                    rhs=stt[:, spos, :],
                                    start=(j == 0), stop=(j == nblk_w - 1))
                                j += 1
                        # mean reciprocal, broadcast to 128 partitions
                        rc = ppool.tile([P, WIN], F32, tag=f"rc{r}", name=f"rc{r}",
                                        space="PSUM", bufs=1)
                        nc.tensor.matmul(
                            out=rc[:], lhsT=ones[:],
                            rhs=recip_sb[:, r * SHP + w * WIN:
                                         r * SHP + (w + 1) * WIN],
                            start=True, stop=True)
                        rcs = wpool.tile([P, WIN], F32, tag=f"rcs{r}", name=f"rcs{r}", bufs=2)
                        nc.scalar.copy(out=rcs[:], in_=rc[:])
                        asb = wpool.tile([P, WIN], BF16, tag=f"asb{r}", name=f"asb{r}", bufs=2)
                        if nblk_w == 0:
                            nc.vector.memset(asb[:], 0.0)
                        else:
                            nc.vector.tensor_tensor(
                                out=asb[:], in0=pa[:], in1=rcs[:],
                                op=mybir.AluOpType.mult)
                        agg.append(asb)

                    po = ppool.tile([P, WIN], F32, tag="po", name="po", space="PSUM")
                    nc.tensor.matmul(out=po[:], lhsT=rootw[:, 0, :],
                                     rhs=h_in[:, ws], start=True, stop=False)
                    for r in range(R):
                        nc.tensor.matmul(out=po[:], lhsT=relw[r][:, 0, :],
                                         rhs=agg[r][:], start=False,
                                         stop=(r == R - 1))
                    nc.scalar.activation(
                        out=h_out[:, ws], in_=po[:],
                        func=mybir.ActivationFunctionType.Identity,
                        bias=bias_t[:, bias_col:bias_col + 1])
                    if emit_next is not None:
                        for blk in range(w * WIN // P, (w + 1) * WIN // P):
                            emit_next(blk)

            # meta (dw table, st-ordered) is layer-independent: load once
            nc.sync.dma_start(out=meta_sb[:], in_=metat[:])

            # table of h0 + layer 1 (emits table-2 blocks as windows finish)
            emit_table(hT[0], 0)
            emit_layer(0, hT[0], hT[1], cc_out[0],
                       wt["root1"], [wt["rel10"], wt["rel11"]], 6,
                       emit_next=lambda blk: emit_table_blk(hT[1], 1, blk))
            # layer 2 (its table chunks AllGathered during layer 1)
            emit_layer(1, hT[1], hT[2], cc_out[1],
                       wt["root2"], [wt["rel20"], wt["rel21"]], 7)

            # =============== classifier ===============
            for w in range(NW):
                ws = slice(w * WIN, (w + 1) * WIN)
                pc = ppool.tile([P, WIN], F32, tag="po", name="pc", space="PSUM")
                nc.tensor.matmul(out=pc[:], lhsT=wt["wcls"][:, 0, :],
                                 rhs=hT[2][:, ws], start=True, stop=True)
                oc = wpool.tile([P, WIN], F32, tag="oc", name="oc", bufs=1)
                nc.scalar.activation(
                    out=oc[:], in_=pc[:],
                    func=mybir.ActivationFunctionType.Identity,
                    bias=bias_t[:, 8:9])
                nc.sync.dma_start(out=outT[:, ws], in_=oc[:])

    nc.compile()
    return nc


# ---------------------------------------------------------------------------
# entry point
# ---------------------------------------------------------------------------

def kernel(**inputs):
    cfg = _derived(CFG)
    return _kernel_impl(inputs, cfg)


def _kernel_impl(inputs, cfg, trace=False):
    d = cfg
    NC, SH, SHP = d["NC"], d["SH"], d["SHP"]

    pl = build_plan(inputs["edge_index"], inputs["edge_type"], d)
    xs = prep_x(np.asarray(inputs["x"], np.float32), d)
    w = prep_weights(inputs, d)

    nc = build_bass(d, pl)

    in_maps = []
    for c in range(NC):
        m = {"xT": xs[c], "idxt": pl.idx16[c], "metat": pl.meta[c],
             "recipt": pl.recip[c], "biases": w["biases"]}
        for nm in ["wdes", "wtweet", "wnum", "wcat", "win", "root1", "rel10",
                   "rel11", "root2", "rel20", "rel21", "wcls"]:
            m[nm] = w[nm]
        in_maps.append(m)

    res = run_bass_kernel_spmd(nc, in_maps, core_ids=list(range(NC)),
                               trace=trace)

    out = np.empty((NC * SH, d["H"]), np.float32)
    for c in range(NC):
        out[c * SH:(c + 1) * SH] = res.results[c]["outT"].T[:SH]
    if trace:
        return out, res
    return out


# revision 16
# speedup vs baseline: 1.0069x; 1.0069x over previous
"""BotRGCN Trainium2 kernel: feature transform + 2 RGCN layers + classifier.

Sharding: nodes split across 8 cores by id (12500/core, padded to 12544).
Edges partitioned by destination shard; per (relation, dst-window, src-bank)
groups padded to a block structure uniform across cores so a single SPMD
program serves all 8 cores. Source features exchanged via bf16 AllGather of
the per-layer node-feature table; gathers via int16 dma_gather per src bank,
spread over 4 SWDGE queues so descriptor generation overlaps ring drain.

Aggregation is a pure sum via one-hot scatter matmuls (one-hot tiles built in
batched DVE is_equal ops); the per-(rel, dst) mean reciprocal is applied
afterwards via a rank-1 broadcast matmul + elementwise multiply.
"""

import sys

sys.path.insert(0, "/opt/trn_rl_repo")

from contextlib import ExitStack

import numpy as np
import ml_dtypes

import concourse.bass as bass
import concourse.bacc as bacc
import concourse.mybir as mybir
import concourse.tile as tile
from concourse.masks import make_identity
from concourse.bass_utils import run_bass_kernel_spmd

BF16 = mybir.dt.bfloat16
F32 = mybir.dt.float32
I16 = mybir.dt.int16

P = 128

# full-problem config (test.py overrides for mini runs)
CFG = dict(
    N=100000,        # nodes
    NC=8,            # cores
    R=2,             # relations
    H=128,
    DES=768, TWEET=768, NUMP=6, CATP=11,
    WIN=256,         # dst window (PSUM free dim)
    NBLK_CH=16,      # gather-chunk size in 128-edge blocks
    SCH=8,           # one-hot build chunk size in blocks
    BANKROWS=25088,  # gather-table bank rows (< 2^15)
    NTF=512,         # feature-stage node tile
)


def _derived(cfg):
    d = dict(cfg)
    d["SH"] = cfg["N"] // cfg["NC"]
    d["SHP"] = ((d["SH"] + P - 1) // P) * P
    d["NW"] = d["SHP"] // cfg["WIN"]
    assert d["SHP"] % cfg["WIN"] == 0
    d["TROWS"] = cfg["NC"] * d["SHP"]           # padded table rows
    # bank-aligned table chunks: per-core rows per chunk (last one short)
    full = (d["SHP"] + 4 - 1) // 4
    full = ((full + P - 1) // P) * P
    chg = []
    left = d["SHP"]
    while left > 0:
        take = min(full, left)
        chg.append(take)
        left -= take
    d["CHG"] = chg
    assert all(c * cfg["NC"] < 2 ** 15 for c in chg)
    d["BANKS"] = len(chg)
    d["TBLK"] = d["SHP"] // P                   # 128-row blobs per core
    # x feature layout: [des | tweet | num(pad to 128) | cat(pad to 128)]
    d["KDES"] = cfg["DES"] // P
    d["KTWEET"] = cfg["TWEET"] // P
    d["KX"] = d["KDES"] + d["KTWEET"] + 2
    d["XROWS"] = d["KX"] * P
    return d


# ---------------------------------------------------------------------------
# host-side graph planning
# ---------------------------------------------------------------------------

class Plan:
    pass


def build_plan(edge_index, edge_type, cfg):
    """Group edges per core by (rel, dst-window, src-bank); pad each group to a
    whole number of 128-edge blocks, uniform across cores. Returns per-core
    gather-index / meta arrays plus the uniform block structure."""
    d = cfg
    NC, SH, SHP, WIN, NW = d["NC"], d["SH"], d["SHP"], d["WIN"], d["NW"]
    BANKS, BR, NBLK_CH = d["BANKS"], d["BANKROWS"], d["NBLK_CH"]
    R = d["R"]
    N = d["N"]
    TBLK = d["TBLK"]

    src = np.asarray(edge_index[0], dtype=np.int64)
    dst = np.asarray(edge_index[1], dtype=np.int64)
    et = np.asarray(edge_type, dtype=np.int64)

    core = dst // SH
    dl = dst - core * SH
    # table layout: 4 bank-aligned chunks; within a chunk, rows are
    # [src-core][node-order]. CHG rows per core per chunk.
    CHG = d["CHG"]
    gof = np.concatenate([[0], np.cumsum(CHG)])       # per-core chunk offsets
    sl = src - (src // SH) * SH
    g = np.minimum(sl // CHG[0], len(CHG) - 1)
    bank = g
    bidx = (gof[g] * 0 + (src // SH) * np.asarray(CHG)[g]
            + (sl - gof[g])).astype(np.int16)
    win = dl // WIN
    dw = (dl - win * WIN).astype(np.float32)

    # per-(rel, node) in-degree -> per-core recip table [R, SHP]
    cnt = np.bincount(et * N + dst, minlength=R * N).reshape(R, N)
    recip_full = (1.0 / np.maximum(cnt, 1.0)).astype(np.float32)   # [R, N]
    recip = np.zeros((NC, R, SHP), np.float32)
    for c in range(NC):
        recip[c, :, :SH] = recip_full[:, c * SH:(c + 1) * SH]

    # group = (bank, win) with edges rel-sorted inside; uniform block counts
    # = max over cores. Slot space is bank-major so each bank is ONE gather
    # stream consumed sequentially by the (w, r) window loop. A block may mix
    # relations near the per-group rel boundary; each (block, rel) pair any
    # core needs gets its own one-hot meta entry (non-target edges stay -1).
    NG = BANKS * NW
    gid = bank * NW + win
    counts = np.bincount(core * NG + gid, minlength=NC * NG).reshape(NC, NG)
    bpg_bw = (counts.max(axis=0) + P - 1) // P         # blocks per group
    bpg_bw = bpg_bw.reshape(BANKS, NW).copy()
    # pad each bank stream to a whole number of chunks (extra blocks to the
    # stream's last group)
    for b in range(BANKS):
        tot = int(bpg_bw[b].sum())
        pad = (-tot) % NBLK_CH
        if tot == 0 and pad == 0:
            pad = NBLK_CH
        bpg_bw[b, NW - 1] += pad

    slots_per_group = (bpg_bw.reshape(-1) * P)
    slot_base = np.zeros(NG + 1, np.int64)
    np.cumsum(slots_per_group, out=slot_base[1:])
    TOTSLOT = int(slot_base[-1])
    TOTBLK = TOTSLOT // P

    # stream bookkeeping: stream = bank; block base per (b, w) group
    group_blk_base = np.zeros((BANKS, NW), np.int64)
    base = 0
    stream_blk_base = np.zeros(BANKS, np.int64)
    for b in range(BANKS):
        stream_blk_base[b] = base
        for w in range(NW):
            group_blk_base[b, w] = base
            base += int(bpg_bw[b, w])
    assert base == TOTBLK

    # per-(core, b, w): r0 edge count -> which blocks each rel touches.
    # needs[r][b][w] = sorted list of block indices k (within the group) that
    # hold rel-r edges on ANY core (always also emit block 0 of rel r's side
    # if group nonempty on any core? no: derive purely from counts).
    cnt0 = np.bincount((core * NG + gid)[et == 0], minlength=NC * NG)
    cnt0 = cnt0.reshape(NC, NG).reshape(NC, BANKS, NW)
    cnt01 = counts.reshape(NC, BANKS, NW)
    needs = [[[None] * NW for _ in range(BANKS)] for _ in range(R)]
    for b in range(BANKS):
        for w in range(NW):
            nb = int(bpg_bw[b, w])
            n0 = cnt0[:, b, w]
            n01 = cnt01[:, b, w]
            k0 = set()
            k1 = set()
            for c in range(NC):
                if n0[c] > 0:
                    k0.update(range(0, (int(n0[c]) + P - 1) // P))
                if n01[c] > int(n0[c]):
                    k1.update(range(int(n0[c]) // P, (int(n01[c]) + P - 1) // P))
            needs[0][b][w] = sorted(k0)
            needs[1][b][w] = sorted(k1)

    # st order: (w, r, b, needed-k) — the one-hot consumption order
    st_entry = {}
    sbase = 0
    st_group_base = np.zeros((NW, R, BANKS), np.int64)
    for w in range(NW):
        for r in range(R):
            for b in range(BANKS):
                st_group_base[w, r, b] = sbase
                for k in needs[r][b][w]:
                    st_entry[(r, b, w, k)] = sbase
                    sbase += 1
    ST_TOT = ((max(sbase, 1) + d["SCH"] - 1) // d["SCH"]) * d["SCH"]

    # place each edge into its group's slot range (per core), rel-sorted
    okey = (core * NG + gid) * R + et
    order = np.argsort(okey, kind="stable")
    gkey = (core * NG + gid)[order]
    first_of = np.r_[True, gkey[1:] != gkey[:-1]]
    idx_in_run = np.arange(len(gkey)) - np.maximum.accumulate(
        np.where(first_of, np.arange(len(gkey)), 0)
    )
    slot = slot_base[gkey % NG] + idx_in_run

    idx16 = np.zeros((NC, 8 * 16, TOTSLOT // 16), np.int16)
    meta = np.full((NC, P, ST_TOT), -1.0, np.float32)
    ecore = core[order]
    col = slot // 16
    prow = (slot % 16).astype(np.int64)
    for g2 in range(8):
        idx16[ecore, 16 * g2 + prow, col] = bidx[order]
    # meta entry per (rel, block): position of each edge within its block
    eb = np.asarray(bank)[order]
    ew = win[order]
    er = et[order]
    ek = (slot - slot_base[gkey % NG]) // P
    es = np.fromiter(
        (st_entry[(int(er[i]), int(eb[i]), int(ew[i]), int(ek[i]))]
         for i in range(len(order))), np.int64, len(order))
    meta[ecore, slot % P, es] = dw[order]

    pl = Plan()
    pl.idx16 = idx16.reshape(NC, P, TOTSLOT // 16)
    pl.meta = meta.astype(ml_dtypes.bfloat16)
    pl.recip = recip.astype(ml_dtypes.bfloat16)
    pl.TOTBLK = TOTBLK
    pl.ST_TOT = ST_TOT
    pl.needs = needs
    pl.group_blk_base = group_blk_base
    pl.stream_blk_base = stream_blk_base
    pl.st_group_base = st_group_base
    return pl


def prep_x(x, cfg):
    """Per-core transposed bf16 feature blocks [XROWS, SHP]."""
    d = cfg
    NC, SH, SHP = d["NC"], d["SH"], d["SHP"]
    NUMP, TWEET, CATP, DES = d["NUMP"], d["TWEET"], d["CATP"], d["DES"]
    KD, KT = d["KDES"], d["KTWEET"]
    out = np.zeros((NC, d["XROWS"], SHP), ml_dtypes.bfloat16)
    for c in range(NC):
        xs = x[c * SH:(c + 1) * SH]
        xT = np.zeros((d["XROWS"], SHP), np.float32)
        xT[:DES, :SH] = xs[:, NUMP + TWEET + CATP:].T
        xT[DES:DES + TWEET, :SH] = xs[:, NUMP:NUMP + TWEET].T
        xT[(KD + KT) * P:(KD + KT) * P + NUMP, :SH] = xs[:, :NUMP].T
        xT[(KD + KT + 1) * P:(KD + KT + 1) * P + CATP, :SH] = \
            xs[:, NUMP + TWEET:NUMP + TWEET + CATP].T
        out[c] = xT.astype(ml_dtypes.bfloat16)
    return out


def prep_weights(inp, cfg):
    """bf16 weight blocks + packed fp32 biases."""
    bf = lambda a: np.asarray(a, np.float32).astype(ml_dtypes.bfloat16)
    d = cfg
    wnum = np.zeros((P, d["H"]), np.float32)
    wnum[:d["NUMP"]] = inp["W_num"]
    wcat = np.zeros((P, d["H"]), np.float32)
    wcat[:d["CATP"]] = inp["W_cat"]
    w = {
        "wdes": bf(inp["W_des"]), "wtweet": bf(inp["W_tweet"]),
        "wnum": bf(wnum), "wcat": bf(wcat), "win": bf(inp["W_in"]),
        "root1": bf(inp["root1"]), "rel10": bf(inp["rel1"][0]),
        "rel11": bf(inp["rel1"][1]),
        "root2": bf(inp["root2"]), "rel20": bf(inp["rel2"][0]),
        "rel21": bf(inp["rel2"][1]), "wcls": bf(inp["W_cls"]),
    }
    biases = np.stack(
        [inp["b_des"], inp["b_tweet"], inp["b_num"], inp["b_cat"],
         inp["b_in"], inp["prelu_a"], inp["bias1"], inp["bias2"],
         inp["b_cls"]], axis=1).astype(np.float32)   # [128, 9]
    w["biases"] = biases
    return w


# ---------------------------------------------------------------------------
# bass program
# ---------------------------------------------------------------------------

def build_bass(cfg, pl):
    d = cfg
    NC, SHP, WIN, NW, NTF = d["NC"], d["SHP"], d["WIN"], d["NW"], d["NTF"]
    BANKS, BR, NBLK_CH = d["BANKS"], d["BANKROWS"], d["NBLK_CH"]
    R, H = d["R"], d["H"]
    KD, KT, KX = d["KDES"], d["KTWEET"], d["KX"]
    TBLK = d["TBLK"]
    TROWS = d["TROWS"]
    SCH = d["SCH"]
    CHS = NBLK_CH * P      # idx slots per chunk

    nc = bacc.Bacc(None, target_bir_lowering=False, debug=False,
                   num_devices=NC, num_swdge_queues=4,
                   dynamic_dma_scratch_size=32768)

    # ---- I/O ----
    xT = nc.dram_tensor("xT", [d["XROWS"], SHP], BF16, kind="ExternalInput")
    idxt = nc.dram_tensor("idxt", [P, pl.TOTBLK * P // 16], I16, kind="ExternalInput")
    metat = nc.dram_tensor("metat", [P, pl.ST_TOT], BF16, kind="ExternalInput")
    recipt = nc.dram_tensor("recipt", [R, SHP], BF16, kind="ExternalInput")
    wts = {}
    for nm, shp in [("wdes", [d["DES"], H]), ("wtweet", [d["TWEET"], H]),
                    ("wnum", [P, H]), ("wcat", [P, H]), ("win", [4 * P, H]),
                    ("root1", [H, H]), ("rel10", [H, H]), ("rel11", [H, H]),
                    ("root2", [H, H]), ("rel20", [H, H]), ("rel21", [H, H]),
                    ("wcls", [H, H])]:
        wts[nm] = nc.dram_tensor(nm, shp, BF16, kind="ExternalInput")
    biases = nc.dram_tensor("biases", [P, 9], F32, kind="ExternalInput")
    outT = nc.dram_tensor("outT", [P, SHP], F32, kind="ExternalOutput")

    # ---- collective tables (bank-aligned chunks) ----
    CHG = d["CHG"]
    cc_in = [[nc.dram_tensor(f"cc{i}_in{g}", [CHG[g], H], BF16,
                             kind="Internal")
              for g in range(len(CHG))] for i in (1, 2)]
    cc_out = [[nc.dram_tensor(f"cc{i}_out{g}", [NC * CHG[g], H], BF16,
                              kind="Internal", addr_space="Shared")
               for g in range(len(CHG))] for i in (1, 2)]

    rg = [list(range(NC))]

    with tile.TileContext(nc) as tc:
        with (
            tc.tile_pool(name="const", bufs=1) as cpool,
            tc.tile_pool(name="resident", bufs=1) as rpool,
            ExitStack() as mstack,
        ):
            # ---- constants ----
            ident = cpool.tile([P, P], BF16)
            make_identity(nc, ident[:])
            # replicated iota: value at (nb, w) = w
            iota = cpool.tile([P, SCH, WIN], BF16)
            nc.gpsimd.iota(iota[:], pattern=[[0, SCH], [1, WIN]], base=0,
                           channel_multiplier=0,
                           allow_small_or_imprecise_dtypes=True)
            ones = cpool.tile([1, P], BF16)
            nc.vector.memset(ones[:], 1.0)
            bias_t = cpool.tile([P, 9], F32)
            nc.sync.dma_start(out=bias_t[:], in_=biases[:])
            recip_sb = cpool.tile([1, R * SHP], BF16)
            nc.sync.dma_start(
                out=recip_sb[:], in_=recipt.rearrange("r n -> (r n)")
                .unsqueeze(0))

            wt = {}
            for nm, kb in [("wdes", KD), ("wtweet", KT), ("wnum", 1),
                           ("wcat", 1), ("win", 4), ("root1", 1),
                           ("rel10", 1), ("rel11", 1), ("root2", 1),
                           ("rel20", 1), ("rel21", 1), ("wcls", 1)]:
                t = cpool.tile([P, kb, H], BF16, tag=f"w_{nm}", name=f"w_{nm}")
                nc.sync.dma_start(
                    out=t[:], in_=wts[nm].rearrange("(k p) h -> p k h", p=P))
                wt[nm] = t

            # resident activations (transposed, [H, SHP] bf16)
            hT = [rpool.tile([P, SHP], BF16, tag="ht", name=f"hT{i}", bufs=2)
                  for i in range(3)]

            # =============== feature transform ===============
            fstack = ExitStack()
            fpool = fstack.enter_context(tc.tile_pool(name="featsb", bufs=2))
            fpp = fstack.enter_context(
                tc.tile_pool(name="featps", bufs=2, space="PSUM"))
            ntiles = (SHP + NTF - 1) // NTF
            for t in range(ntiles):
                n0 = t * NTF
                n1 = min(SHP, n0 + NTF)
                nn = n1 - n0
                xt = fpool.tile([P, KX, NTF], BF16, tag="xt", name="xt")
                nc.sync.dma_start(
                    out=xt[:, :, :nn],
                    in_=xT.rearrange("(k p) n -> p k n", p=P)[:, :, n0:n1])

                zb = []
                for bi, (wnm, ks, kn) in enumerate([
                        ("wdes", 0, KD), ("wtweet", KD, KT),
                        ("wnum", KD + KT, 1), ("wcat", KD + KT + 1, 1)]):
                    pz = fpp.tile([P, NTF], F32, tag=f"pz{bi}", name=f"pz{bi}", space="PSUM", bufs=1)
                    for k in range(kn):
                        nc.tensor.matmul(
                            out=pz[:, :nn], lhsT=wt[wnm][:, k, :],
                            rhs=xt[:, ks + k, :nn],
                            start=(k == 0), stop=(k == kn - 1))
                    v = fpool.tile([P, NTF], BF16, tag=f"v{bi}", name=f"v{bi}")
                    nc.scalar.activation(
                        out=v[:, :nn], in_=pz[:, :nn],
                        func=mybir.ActivationFunctionType.Identity,
                        bias=bias_t[:, bi:bi + 1])
                    z = fpool.tile([P, NTF], BF16, tag=f"z{bi}", name=f"z{bi}")
                    nc.vector.scalar_tensor_tensor(
                        out=z[:, :nn], in0=v[:, :nn], scalar=0.01,
                        in1=v[:, :nn], op0=mybir.AluOpType.mult,
                        op1=mybir.AluOpType.max)
                    zb.append(z)

                ph = fpp.tile([P, NTF], F32, tag="ph", name="ph", space="PSUM")
                for k in range(4):
                    nc.tensor.matmul(out=ph[:, :nn], lhsT=wt["win"][:, k, :],
                                     rhs=zb[k][:, :nn],
                                     start=(k == 0), stop=(k == 3))
                vh = fpool.tile([P, NTF], F32, tag="vh", name="vh")
                nc.scalar.activation(
                    out=vh[:, :nn], in_=ph[:, :nn],
                    func=mybir.ActivationFunctionType.Identity,
                    bias=bias_t[:, 4:5])
                nc.vector.scalar_tensor_tensor(
                    out=hT[0][:, n0:n1], in0=vh[:, :nn],
                    scalar=bias_t[:, 5:6], in1=vh[:, :nn],
                    op0=mybir.AluOpType.mult, op1=mybir.AluOpType.max)

            fstack.close()
            wpool = mstack.enter_context(tc.tile_pool(name="work", bufs=3))
            ppool = mstack.enter_context(
                tc.tile_pool(name="psum", bufs=2, space="PSUM"))

            # resident meta (dw per st-ordered block), reloaded per layer
            meta_sb = rpool.tile([P, pl.ST_TOT], BF16, tag="meta",
                                 name="meta", bufs=1)

            # =============== per-layer helpers ===============
            chunk_blk0 = [0]
            for g in range(len(CHG)):
                chunk_blk0.append(chunk_blk0[-1] + CHG[g] // P)

            def emit_table_blk(src_hT, li, blk):
                g = 0
                while blk >= chunk_blk0[g + 1]:
                    g += 1
                lb = blk - chunk_blk0[g]
                tp = ppool.tile([P, P], BF16, tag="tp", name="tp", space="PSUM", bufs=2)
                nc.tensor.transpose(
                    out=tp[:], in_=src_hT[:, blk * P:(blk + 1) * P],
                    identity=ident[:])
                rowt = wpool.tile([P, P], BF16, tag="rowt", name="rowt",
                                  bufs=3)
                nc.scalar.copy(out=rowt[:], in_=tp[:])
                nc.sync.dma_start(out=cc_in[li][g][lb * P:(lb + 1) * P, :],
                                  in_=rowt[:])
                if blk + 1 == chunk_blk0[g + 1]:
                    nc.gpsimd.collective_compute(
                        "AllGather", mybir.AluOpType.bypass,
                        ins=[cc_in[li][g][:]], outs=[cc_out[li][g][:]],
                        replica_groups=rg)

            def emit_table(src_hT, li):
                for blk in range(TBLK):
                    emit_table_blk(src_hT, li, blk)

            def emit_layer(li, h_in, h_out, tables, rootw, relw, bias_col,
                           emit_next=None):
                # per-stream gather state
                cur = {}
                cur_st = {}

                def ensure_chunk(b, blkloc):
                    ch = blkloc // NBLK_CH
                    if cur.get(b, (-1,))[0] == ch:
                        return cur[b]
                    gblk0 = int(pl.stream_blk_base[b]) + ch * NBLK_CH
                    it = wpool.tile([P, CHS // 16], I16, tag=f"idx{b}", name=f"idx{b}", bufs=3)
                    nc.sync.dma_start(
                        out=it[:],
                        in_=idxt[:, gblk0 * P // 16:(gblk0 + NBLK_CH) * P // 16])
                    gt = wpool.tile([P, NBLK_CH, P], BF16, tag=f"gt{b}", name=f"gt{b}", bufs=3)
                    nc.gpsimd.dma_gather(
                        out_ap=gt[:],
                        in_ap=tables[b][:],
                        idxs_ap=it[:], num_idxs=CHS, num_idxs_reg=CHS,
                        elem_size=H, single_packet=False, queue_num=b % 4)
                    cur[b] = (ch, gt)
                    return cur[b]

                def ensure_st(stblk):
                    ch = stblk // SCH
                    if cur_st.get("c", -1) == ch:
                        return cur_st["t"]
                    stt = wpool.tile([P, SCH, WIN], BF16, tag="onehot",
                                     name="onehot", bufs=3)
                    m0 = ch * SCH
                    nc.vector.tensor_tensor(
                        out=stt[:],
                        in0=iota[:],
                        in1=meta_sb[:, m0:m0 + SCH].unsqueeze(2)
                            .to_broadcast([P, SCH, WIN]),
                        op=mybir.AluOpType.is_equal)
                    cur_st["c"] = ch
                    cur_st["t"] = stt
                    return stt

                for w in range(NW):
                    ws = slice(w * WIN, (w + 1) * WIN)
                    agg = []
                    for r in range(R):
                        pa = ppool.tile([P, WIN], F32, tag=f"agg{r}", name=f"agg{r}",
                                        space="PSUM", bufs=1)
                        nblk_w = sum(len(pl.needs[r][b][w])
                                     for b in range(BANKS))
                        j = 0
                        stbase = int(pl.st_group_base[w, r, 0])
                        for b in range(BANKS):
                            base = int(pl.group_blk_base[b, w]
                                       - pl.stream_blk_base[b])
                            for k in pl.needs[r][b][w]:
                                blkloc = base + k
                                ch, gt = ensure_chunk(b, blkloc)
                                pos = blkloc - ch * NBLK_CH
                                stblk = stbase + j
                                stt = ensure_st(stblk)
                                spos = stblk - (stblk // SCH) * SCH
                                nc.tensor.matmul(
                                    out=pa[:], lhsT=gt[:, pos, :],
                                    rhs=stt[:, spos, :],
                                    start=(j == 0), stop=(j == nblk_w - 1))
                                j += 1
                        # mean reciprocal, broadcast to 128 partitions
                        rc = ppool.tile([P, WIN], F32, tag=f"rc{r}", name=f"rc{r}",
                                        space="PSUM", bufs=1)
                        nc.tensor.matmul(
                            out=rc[:], lhsT=ones[:],
                            rhs=recip_sb[:, r * SHP + w * WIN:
                                         r * SHP + (w + 1) * WIN],
                            start=True, stop=True)
                        rcs = wpool.tile([P, WIN], F32, tag=f"rcs{r}", name=f"rcs{r}", bufs=2)
                        nc.scalar.copy(out=rcs[:], in_=rc[:])
                        asb = wpool.tile([P, WIN], BF16, tag=f"asb{r}", name=f"asb{r}", bufs=2)
                        if nblk_w == 0:
                            nc.vector.memset(asb[:], 0.0)
                        else:
                            nc.vector.tensor_tensor(
                                out=asb[:], in0=pa[:], in1=rcs[:],
                                op=mybir.AluOpType.mult)
                        agg.append(asb)

                    po = ppool.tile([P, WIN], F32, tag="po", name="po", space="PSUM")
                    nc.tensor.matmul(out=po[:], lhsT=rootw[:, 0, :],
                                     rhs=h_in[:, ws], start=True, stop=False)
                    for r in range(R):
                        nc.tensor.matmul(out=po[:], lhsT=relw[r][:, 0, :],
                                         rhs=agg[r][:], start=False,
                                         stop=(r == R - 1))
                    nc.scalar.activation(
                        out=h_out[:, ws], in_=po[:],
                        func=mybir.ActivationFunctionType.Identity,
                        bias=bias_t[:, bias_col:bias_col + 1])
                    if emit_next is not None:
                        for blk in range(w * WIN // P, (w + 1) * WIN // P):
                            emit_next(blk)

            # meta (dw table, st-ordered) is layer-independent: load once
            nc.sync.dma_start(out=meta_sb[:], in_=metat[:])

            # table of h0 + layer 1
            emit_table(hT[0], 0)
            emit_layer(0, hT[0], hT[1], cc_out[0],
                       wt["root1"], [wt["rel10"], wt["rel11"]], 6)
            # table of h1 + layer 2
            emit_table(hT[1], 1)
            emit_layer(1, hT[1], hT[2], cc_out[1],
                       wt["root2"], [wt["rel20"], wt["rel21"]], 7)

            # =============== classifier ===============
            for w in range(NW):
                ws = slice(w * WIN, (w + 1) * WIN)
                pc = ppool.tile([P, WIN], F32, tag="po", name="pc", space="PSUM")
                nc.tensor.matmul(out=pc[:], lhsT=wt["wcls"][:, 0, :],
                                 rhs=hT[2][:, ws], start=True, stop=True)
                oc = wpool.tile([P, WIN], F32, tag="oc", name="oc", bufs=1)
                nc.scalar.activation(
                    out=oc[:], in_=pc[:],
                    func=mybir.ActivationFunctionType.Identity,
                    bias=bias_t[:, 8:9])
                nc.sync.dma_start(out=outT[:, ws], in_=oc[:])

    nc.compile()
    return nc


# ---------------------------------------------------------------------------
# entry point
# ---------------------------------------------------------------------------

def kernel(**inputs):
    cfg = _derived(CFG)
    return _kernel_impl(inputs, cfg)


def _kernel_impl(inputs, cfg, trace=False):
    d = cfg
    NC, SH, SHP = d["NC"], d["SH"], d["SHP"]

    pl = build_plan(inputs["edge_index"], inputs["edge_type"], d)
    xs = prep_x(np.asarray(inputs["x"], np.float32), d)
    w = prep_weights(inputs, d)

    nc = build_bass(d, pl)

    in_maps = []
    for c in range(NC):
        m = {"xT": xs[c], "idxt": pl.idx16[c], "metat": pl.meta[c],
             "recipt": pl.recip[c], "biases": w["biases"]}
        for nm in ["wdes", "wtweet", "wnum", "wcat", "win", "root1", "rel10",
                   "rel11", "root2", "rel20", "rel21", "wcls"]:
            m[nm] = w[nm]
        in_maps.append(m)

    res = run_bass_kernel_spmd(nc, in_maps, core_ids=list(range(NC)),
                               trace=trace)

    out = np.empty((NC * SH, d["H"]), np.float32)
    for c in range(NC):
        out[c * SH:(c + 1) * SH] = res.results[c]["outT"].T[:SH]
    if trace:
        return out, res
    return out


# revision 17
# speedup vs baseline: 1.0069x; 1.0000x over previous
"""BotRGCN Trainium2 kernel: feature transform + 2 RGCN layers + classifier.

Sharding: nodes split across 8 cores by id (12500/core, padded to 12544).
Edges partitioned by destination shard; per (relation, dst-window, src-bank)
groups padded to a block structure uniform across cores so a single SPMD
program serves all 8 cores. Source features exchanged via bf16 AllGather of
the per-layer node-feature table; gathers via int16 dma_gather per src bank,
spread over 4 SWDGE queues so descriptor generation overlaps ring drain.

Aggregation is a pure sum via one-hot scatter matmuls (one-hot tiles built in
batched DVE is_equal ops); the per-(rel, dst) mean reciprocal is applied
afterwards via a rank-1 broadcast matmul + elementwise multiply.
"""

import sys

sys.path.insert(0, "/opt/trn_rl_repo")

from contextlib import ExitStack

import numpy as np
import ml_dtypes

import concourse.bass as bass
import concourse.bacc as bacc
import concourse.mybir as mybir
import concourse.tile as tile
from concourse.masks import make_identity
from concourse.bass_utils import run_bass_kernel_spmd

BF16 = mybir.dt.bfloat16
F32 = mybir.dt.float32
I16 = mybir.dt.int16

P = 128

# full-problem config (test.py overrides for mini runs)
CFG = dict(
    N=100000,        # nodes
    NC=8,            # cores
    R=2,             # relations
    H=128,
    DES=768, TWEET=768, NUMP=6, CATP=11,
    WIN=256,         # dst window (PSUM free dim)
    NBLK_CH=16,      # gather-chunk size in 128-edge blocks
    SCH=8,           # one-hot build chunk size in blocks
    BANKROWS=25088,  # gather-table bank rows (< 2^15)
    NTF=512,         # feature-stage node tile
)


def _derived(cfg):
    d = dict(cfg)
    d["SH"] = cfg["N"] // cfg["NC"]
    d["SHP"] = ((d["SH"] + P - 1) // P) * P
    d["NW"] = d["SHP"] // cfg["WIN"]
    assert d["SHP"] % cfg["WIN"] == 0
    d["TROWS"] = cfg["NC"] * d["SHP"]           # padded table rows
    d["BANKS"] = (d["TROWS"] + cfg["BANKROWS"] - 1) // cfg["BANKROWS"]
    d["TBLK"] = d["SHP"] // P                   # 128-row blobs per core
    # x feature layout: [des | tweet | num(pad to 128) | cat(pad to 128)]
    d["KDES"] = cfg["DES"] // P
    d["KTWEET"] = cfg["TWEET"] // P
    d["KX"] = d["KDES"] + d["KTWEET"] + 2
    d["XROWS"] = d["KX"] * P
    return d


# ---------------------------------------------------------------------------
# host-side graph planning
# ---------------------------------------------------------------------------

class Plan:
    pass


def build_plan(edge_index, edge_type, cfg):
    """Group edges per core by (rel, dst-window, src-bank); pad each group to a
    whole number of 128-edge blocks, uniform across cores. Returns per-core
    gather-index / meta arrays plus the uniform block structure."""
    d = cfg
    NC, SH, SHP, WIN, NW = d["NC"], d["SH"], d["SHP"], d["WIN"], d["NW"]
    BANKS, BR, NBLK_CH = d["BANKS"], d["BANKROWS"], d["NBLK_CH"]
    R = d["R"]
    N = d["N"]
    TBLK = d["TBLK"]

    src = np.asarray(edge_index[0], dtype=np.int64)
    dst = np.asarray(edge_index[1], dtype=np.int64)
    et = np.asarray(edge_type, dtype=np.int64)

    core = dst // SH
    dl = dst - core * SH
    # table row of a (padded) node: blob layout [p][t] per core
    sl = src - (src // SH) * SH
    ps = (src // SH) * SHP + (sl % P) * TBLK + (sl // P)
    bank = ps // BR
    bidx = (ps - bank * BR).astype(np.int16)
    win = dl // WIN
    dw = (dl - win * WIN).astype(np.float32)

    # per-(rel, node) in-degree -> per-core recip table [R, SHP]
    cnt = np.bincount(et * N + dst, minlength=R * N).reshape(R, N)
    recip_full = (1.0 / np.maximum(cnt, 1.0)).astype(np.float32)   # [R, N]
    recip = np.zeros((NC, R, SHP), np.float32)
    for c in range(NC):
        recip[c, :, :SH] = recip_full[:, c * SH:(c + 1) * SH]

    # group = (bank | win, rel); uniform block counts = max over cores.
    # Slot space is bank-major so each bank is ONE gather stream consumed
    # sequentially by the (w, r) window loop.
    NG = R * BANKS * NW
    gid = (bank * NW + win) * R + et
    counts = np.bincount(core * NG + gid, minlength=NC * NG).reshape(NC, NG)
    bpg_bwr = (counts.max(axis=0) + P - 1) // P        # blocks per group
    bpg_bwr = bpg_bwr.reshape(BANKS, NW, R).copy()
    # pad each bank stream to a whole number of chunks (extra blocks to the
    # stream's last group)
    for b in range(BANKS):
        tot = int(bpg_bwr[b].sum())
        pad = (-tot) % NBLK_CH
        if tot == 0 and pad == 0:
            pad = NBLK_CH  # keep streams non-empty for uniform structure
        bpg_bwr[b, NW - 1, R - 1] += pad
    bpg = np.transpose(bpg_bwr, (2, 0, 1))             # [R, BANKS, NW] view

    slots_per_group = (bpg_bwr.reshape(-1) * P)
    slot_base = np.zeros(NG + 1, np.int64)
    np.cumsum(slots_per_group, out=slot_base[1:])
    TOTSLOT = int(slot_base[-1])
    TOTBLK = TOTSLOT // P

    # stream bookkeeping: stream = bank; block base per (b, w, r) group
    group_blk_base = np.zeros((BANKS, NW, R), np.int64)
    base = 0
    stream_blk_base = np.zeros(BANKS, np.int64)
    stream_nblk = np.zeros(BANKS, np.int64)
    for b in range(BANKS):
        stream_blk_base[b] = base
        for w in range(NW):
            for r in range(R):
                group_blk_base[b, w, r] = base
                base += int(bpg_bwr[b, w, r])
        stream_nblk[b] = base - stream_blk_base[b]
    assert base == TOTBLK

    # st order: blocks sorted by (w, r, b, k) — the one-hot consumption order
    st_blk_base = np.zeros((NW, R, BANKS), np.int64)
    sbase = 0
    for w in range(NW):
        for r in range(R):
            for b in range(BANKS):
                st_blk_base[w, r, b] = sbase
                sbase += int(bpg_bwr[b, w, r])
    assert sbase == TOTBLK
    # map stream-order block -> st-order block
    stream2st = np.zeros(TOTBLK, np.int64)
    for b in range(BANKS):
        for w in range(NW):
            for r in range(R):
                gb = group_blk_base[b, w, r]
                sb = st_blk_base[w, r, b]
                for k in range(int(bpg_bwr[b, w, r])):
                    stream2st[gb + k] = sb + k

    # place each edge into its group's slot range (per core)
    okey = core * NG + gid
    order = np.argsort(okey, kind="stable")
    so = okey[order]
    first_of = np.r_[True, so[1:] != so[:-1]]
    idx_in_run = np.arange(len(so)) - np.maximum.accumulate(
        np.where(first_of, np.arange(len(so)), 0)
    )
    slot = slot_base[so % NG] + idx_in_run

    idx16 = np.zeros((NC, 8 * 16, TOTSLOT // 16), np.int16)
    # dw meta in st-order, bf16, padding slots get -1 (never matches iota)
    meta = np.full((NC, P, TOTBLK), -1.0, np.float32)
    ecore = core[order]
    col = slot // 16
    prow = (slot % 16).astype(np.int64)
    for g in range(8):
        idx16[ecore, 16 * g + prow, col] = bidx[order]
    st_slot = stream2st[slot // P] * P + (slot % P)
    meta[ecore, st_slot % P, st_slot // P] = dw[order]

    pl = Plan()
    pl.idx16 = idx16.reshape(NC, P, TOTSLOT // 16)
    pl.meta = meta.astype(ml_dtypes.bfloat16)
    pl.recip = recip.astype(ml_dtypes.bfloat16)
    pl.bpg = bpg
    pl.bpg_bwr = bpg_bwr
    pl.TOTBLK = TOTBLK
    pl.group_blk_base = group_blk_base
    pl.stream_blk_base = stream_blk_base
    pl.stream_nblk = stream_nblk
    pl.st_blk_base = st_blk_base
    return pl


def prep_x(x, cfg):
    """Per-core transposed bf16 feature blocks [XROWS, SHP]."""
    d = cfg
    NC, SH, SHP = d["NC"], d["SH"], d["SHP"]
    NUMP, TWEET, CATP, DES = d["NUMP"], d["TWEET"], d["CATP"], d["DES"]
    KD, KT = d["KDES"], d["KTWEET"]
    out = np.zeros((NC, d["XROWS"], SHP), ml_dtypes.bfloat16)
    for c in range(NC):
        xs = x[c * SH:(c + 1) * SH]
        xT = np.zeros((d["XROWS"], SHP), np.float32)
        xT[:DES, :SH] = xs[:, NUMP + TWEET + CATP:].T
        xT[DES:DES + TWEET, :SH] = xs[:, NUMP:NUMP + TWEET].T
        xT[(KD + KT) * P:(KD + KT) * P + NUMP, :SH] = xs[:, :NUMP].T
        xT[(KD + KT + 1) * P:(KD + KT + 1) * P + CATP, :SH] = \
            xs[:, NUMP + TWEET:NUMP + TWEET + CATP].T
        out[c] = xT.astype(ml_dtypes.bfloat16)
    return out


def prep_weights(inp, cfg):
    """bf16 weight blocks + packed fp32 biases."""
    bf = lambda a: np.asarray(a, np.float32).astype(ml_dtypes.bfloat16)
    d = cfg
    wnum = np.zeros((P, d["H"]), np.float32)
    wnum[:d["NUMP"]] = inp["W_num"]
    wcat = np.zeros((P, d["H"]), np.float32)
    wcat[:d["CATP"]] = inp["W_cat"]
    w = {
        "wdes": bf(inp["W_des"]), "wtweet": bf(inp["W_tweet"]),
        "wnum": bf(wnum), "wcat": bf(wcat), "win": bf(inp["W_in"]),
        "root1": bf(inp["root1"]), "rel10": bf(inp["rel1"][0]),
        "rel11": bf(inp["rel1"][1]),
        "root2": bf(inp["root2"]), "rel20": bf(inp["rel2"][0]),
        "rel21": bf(inp["rel2"][1]), "wcls": bf(inp["W_cls"]),
    }
    biases = np.stack(
        [inp["b_des"], inp["b_tweet"], inp["b_num"], inp["b_cat"],
         inp["b_in"], inp["prelu_a"], inp["bias1"], inp["bias2"],
         inp["b_cls"]], axis=1).astype(np.float32)   # [128, 9]
    w["biases"] = biases
    return w


# ---------------------------------------------------------------------------
# bass program
# ---------------------------------------------------------------------------

def build_bass(cfg, pl):
    d = cfg
    NC, SHP, WIN, NW, NTF = d["NC"], d["SHP"], d["WIN"], d["NW"], d["NTF"]
    BANKS, BR, NBLK_CH = d["BANKS"], d["BANKROWS"], d["NBLK_CH"]
    R, H = d["R"], d["H"]
    KD, KT, KX = d["KDES"], d["KTWEET"], d["KX"]
    TBLK = d["TBLK"]
    TROWS = d["TROWS"]
    SCH = d["SCH"]
    CHS = NBLK_CH * P      # idx slots per chunk

    nc = bacc.Bacc(None, target_bir_lowering=False, debug=False,
                   num_devices=NC, num_swdge_queues=4,
                   dynamic_dma_scratch_size=32768)

    # ---- I/O ----
    xT = nc.dram_tensor("xT", [d["XROWS"], SHP], BF16, kind="ExternalInput")
    idxt = nc.dram_tensor("idxt", [P, pl.TOTBLK * P // 16], I16, kind="ExternalInput")
    metat = nc.dram_tensor("metat", [P, pl.TOTBLK], BF16, kind="ExternalInput")
    recipt = nc.dram_tensor("recipt", [R, SHP], BF16, kind="ExternalInput")
    wts = {}
    for nm, shp in [("wdes", [d["DES"], H]), ("wtweet", [d["TWEET"], H]),
                    ("wnum", [P, H]), ("wcat", [P, H]), ("win", [4 * P, H]),
                    ("root1", [H, H]), ("rel10", [H, H]), ("rel11", [H, H]),
                    ("root2", [H, H]), ("rel20", [H, H]), ("rel21", [H, H]),
                    ("wcls", [H, H])]:
        wts[nm] = nc.dram_tensor(nm, shp, BF16, kind="ExternalInput")
    biases = nc.dram_tensor("biases", [P, 9], F32, kind="ExternalInput")
    outT = nc.dram_tensor("outT", [P, SHP], F32, kind="ExternalOutput")

    # ---- collective tables ----
    cc_in = [nc.dram_tensor(f"cc{i}_in", [SHP, H], BF16, kind="Internal")
             for i in (1, 2)]
    cc_out = [nc.dram_tensor(f"cc{i}_out", [NC * SHP, H], BF16,
                             kind="Internal", addr_space="Shared")
              for i in (1, 2)]

    rg = [list(range(NC))]

    with tile.TileContext(nc) as tc:
        with (
            tc.tile_pool(name="const", bufs=1) as cpool,
            tc.tile_pool(name="resident", bufs=1) as rpool,
            ExitStack() as mstack,
        ):
            # ---- constants ----
            ident = cpool.tile([P, P], BF16)
            make_identity(nc, ident[:])
            # replicated iota: value at (nb, w) = w
            iota = cpool.tile([P, SCH, WIN], BF16)
            nc.gpsimd.iota(iota[:], pattern=[[0, SCH], [1, WIN]], base=0,
                           channel_multiplier=0,
                           allow_small_or_imprecise_dtypes=True)
            ones = cpool.tile([1, P], BF16)
            nc.vector.memset(ones[:], 1.0)
            bias_t = cpool.tile([P, 9], F32)
            nc.sync.dma_start(out=bias_t[:], in_=biases[:])
            recip_sb = cpool.tile([1, R * SHP], BF16)
            nc.sync.dma_start(
                out=recip_sb[:], in_=recipt.rearrange("r n -> (r n)")
                .unsqueeze(0))

            wt = {}
            for nm, kb in [("wdes", KD), ("wtweet", KT), ("wnum", 1),
                           ("wcat", 1), ("win", 4), ("root1", 1),
                           ("rel10", 1), ("rel11", 1), ("root2", 1),
                           ("rel20", 1), ("rel21", 1), ("wcls", 1)]:
                t = cpool.tile([P, kb, H], BF16, tag=f"w_{nm}", name=f"w_{nm}")
                nc.sync.dma_start(
                    out=t[:], in_=wts[nm].rearrange("(k p) h -> p k h", p=P))
                wt[nm] = t

            # resident activations (transposed, [H, SHP] bf16)
            hT = [rpool.tile([P, SHP], BF16, tag="ht", name=f"hT{i}", bufs=2)
                  for i in range(3)]

            # =============== feature transform ===============
            fstack = ExitStack()
            fpool = fstack.enter_context(tc.tile_pool(name="featsb", bufs=2))
            fpp = fstack.enter_context(
                tc.tile_pool(name="featps", bufs=2, space="PSUM"))
            ntiles = (SHP + NTF - 1) // NTF
            for t in range(ntiles):
                n0 = t * NTF
                n1 = min(SHP, n0 + NTF)
                nn = n1 - n0
                xt = fpool.tile([P, KX, NTF], BF16, tag="xt", name="xt")
                nc.sync.dma_start(
                    out=xt[:, :, :nn],
                    in_=xT.rearrange("(k p) n -> p k n", p=P)[:, :, n0:n1])

                zb = []
                for bi, (wnm, ks, kn) in enumerate([
                        ("wdes", 0, KD), ("wtweet", KD, KT),
                        ("wnum", KD + KT, 1), ("wcat", KD + KT + 1, 1)]):
                    pz = fpp.tile([P, NTF], F32, tag=f"pz{bi}", name=f"pz{bi}", space="PSUM", bufs=1)
                    for k in range(kn):
                        nc.tensor.matmul(
                            out=pz[:, :nn], lhsT=wt[wnm][:, k, :],
                            rhs=xt[:, ks + k, :nn],
                            start=(k == 0), stop=(k == kn - 1))
                    v = fpool.tile([P, NTF], BF16, tag=f"v{bi}", name=f"v{bi}")
                    nc.scalar.activation(
                        out=v[:, :nn], in_=pz[:, :nn],
                        func=mybir.ActivationFunctionType.Identity,
                        bias=bias_t[:, bi:bi + 1])
                    z = fpool.tile([P, NTF], BF16, tag=f"z{bi}", name=f"z{bi}")
                    nc.vector.scalar_tensor_tensor(
                        out=z[:, :nn], in0=v[:, :nn], scalar=0.01,
                        in1=v[:, :nn], op0=mybir.AluOpType.mult,
                        op1=mybir.AluOpType.max)
                    zb.append(z)

                ph = fpp.tile([P, NTF], F32, tag="ph", name="ph", space="PSUM")
                for k in range(4):
                    nc.tensor.matmul(out=ph[:, :nn], lhsT=wt["win"][:, k, :],
                                     rhs=zb[k][:, :nn],
                                     start=(k == 0), stop=(k == 3))
                vh = fpool.tile([P, NTF], F32, tag="vh", name="vh")
                nc.scalar.activation(
                    out=vh[:, :nn], in_=ph[:, :nn],
                    func=mybir.ActivationFunctionType.Identity,
                    bias=bias_t[:, 4:5])
                nc.vector.scalar_tensor_tensor(
                    out=hT[0][:, n0:n1], in0=vh[:, :nn],
                    scalar=bias_t[:, 5:6], in1=vh[:, :nn],
                    op0=mybir.AluOpType.mult, op1=mybir.AluOpType.max)

            fstack.close()
            wpool = mstack.enter_context(tc.tile_pool(name="work", bufs=3))
            ppool = mstack.enter_context(
                tc.tile_pool(name="psum", bufs=2, space="PSUM"))

            # resident meta (dw per st-ordered block), reloaded per layer
            meta_sb = rpool.tile([P, pl.TOTBLK], BF16, tag="meta",
                                 name="meta", bufs=1)

            # =============== per-layer helper ===============
            def emit_table(src_hT, cc_in_t, cc_out_t):
                cc_v = cc_in_t.rearrange("(p t) h -> p t h", p=P)
                for blk in range(TBLK):
                    tp = ppool.tile([P, P], BF16, tag="tp", name="tp", space="PSUM", bufs=2)
                    nc.tensor.transpose(
                        out=tp[:], in_=src_hT[:, blk * P:(blk + 1) * P],
                        identity=ident[:])
                    rowt = wpool.tile([P, P], BF16, tag="rowt", name="rowt",
                                      bufs=3)
                    nc.scalar.copy(out=rowt[:], in_=tp[:])
                    nc.sync.dma_start(out=cc_v[:, blk, :], in_=rowt[:])
                nc.gpsimd.collective_compute(
                    "AllGather", mybir.AluOpType.bypass,
                    ins=[cc_in_t[:]], outs=[cc_out_t[:]], replica_groups=rg)

            def emit_layer(li, h_in, h_out, table, rootw, relw, bias_col):
                # fresh meta for this layer (dw table, st-ordered)
                nc.sync.dma_start(out=meta_sb[:], in_=metat[:])

                # per-stream gather state
                cur = {}
                cur_st = {}

                def ensure_chunk(b, blkloc):
                    ch = blkloc // NBLK_CH
                    if cur.get(b, (-1,))[0] == ch:
                        return cur[b]
                    gblk0 = int(pl.stream_blk_base[b]) + ch * NBLK_CH
                    it = wpool.tile([P, CHS // 16], I16, tag=f"idx{b}", name=f"idx{b}", bufs=3)
                    nc.sync.dma_start(
                        out=it[:],
                        in_=idxt[:, gblk0 * P // 16:(gblk0 + NBLK_CH) * P // 16])
                    gt = wpool.tile([P, NBLK_CH, P], BF16, tag=f"gt{b}", name=f"gt{b}", bufs=3)
                    nc.gpsimd.dma_gather(
                        out_ap=gt[:],
                        in_ap=table[b * BR:min((b + 1) * BR, TROWS), :],
                        idxs_ap=it[:], num_idxs=CHS, num_idxs_reg=CHS,
                        elem_size=H, single_packet=False, queue_num=b % 4)
                    cur[b] = (ch, gt)
                    return cur[b]

                def ensure_st(stblk):
                    ch = stblk // SCH
                    if cur_st.get("c", -1) == ch:
                        return cur_st["t"]
                    stt = wpool.tile([P, SCH, WIN], BF16, tag="onehot",
                                     name="onehot", bufs=3)
                    m0 = ch * SCH
                    nc.vector.tensor_tensor(
                        out=stt[:],
                        in0=meta_sb[:, m0:m0 + SCH].unsqueeze(2)
                            .to_broadcast([P, SCH, WIN]),
                        in1=iota[:],
                        op=mybir.AluOpType.is_equal)
                    cur_st["c"] = ch
                    cur_st["t"] = stt
                    return stt

                for w in range(NW):
                    ws = slice(w * WIN, (w + 1) * WIN)
                    agg = []
                    for r in range(R):
                        pa = ppool.tile([P, WIN], F32, tag=f"agg{r}", name=f"agg{r}",
                                        space="PSUM", bufs=1)
                        nblk_w = int(pl.bpg_bwr[:, w, r].sum())
                        j = 0
                        st0 = int(pl.st_blk_base[w, r, 0])
                        for b in range(BANKS):
                            base = int(pl.group_blk_base[b, w, r]
                                       - pl.stream_blk_base[b])
                            for k in range(int(pl.bpg_bwr[b, w, r])):
                                blkloc = base + k
                                ch, gt = ensure_chunk(b, blkloc)
                                pos = blkloc - ch * NBLK_CH
                                stblk = st0 + j
                                stt = ensure_st(stblk)
                                spos = stblk - (stblk // SCH) * SCH
                                nc.tensor.matmul(
                                    out=pa[:], lhsT=gt[:, pos, :],
                                    rhs=stt[:, spos, :],
                                    start=(j == 0), stop=(j == nblk_w - 1))
                                j += 1
                        # mean reciprocal, broadcast to 128 partitions
                        rc = ppool.tile([P, WIN], F32, tag=f"rc{r}", name=f"rc{r}",
                                        space="PSUM", bufs=1)
                        nc.tensor.matmul(
                            out=rc[:], lhsT=ones[:],
                            rhs=recip_sb[:, r * SHP + w * WIN:
                                         r * SHP + (w + 1) * WIN],
                            start=True, stop=True)
                        rcs = wpool.tile([P, WIN], F32, tag=f"rcs{r}", name=f"rcs{r}", bufs=2)
                        nc.scalar.copy(out=rcs[:], in_=rc[:])
                        asb = wpool.tile([P, WIN], BF16, tag=f"asb{r}", name=f"asb{r}", bufs=2)
                        if nblk_w == 0:
                            nc.vector.memset(asb[:], 0.0)
                        else:
                            nc.vector.tensor_tensor(
                                out=asb[:], in0=pa[:], in1=rcs[:],
                                op=mybir.AluOpType.mult)
                        agg.append(asb)

                    po = ppool.tile([P, WIN], F32, tag="po", name="po", space="PSUM")
                    nc.tensor.matmul(out=po[:], lhsT=rootw[:, 0, :],
                                     rhs=h_in[:, ws], start=True, stop=False)
                    for r in range(R):
                        nc.tensor.matmul(out=po[:], lhsT=relw[r][:, 0, :],
                                         rhs=agg[r][:], start=False,
                                         stop=(r == R - 1))
                    nc.scalar.activation(
                        out=h_out[:, ws], in_=po[:],
                        func=mybir.ActivationFunctionType.Identity,
                        bias=bias_t[:, bias_col:bias_col + 1])

            # table of h0 + layer 1
            emit_table(hT[0], cc_in[0], cc_out[0])
            emit_layer(0, hT[0], hT[1], cc_out[0],
                       wt["root1"], [wt["rel10"], wt["rel11"]], 6)
            # table of h1 + layer 2
            emit_table(hT[1], cc_in[1], cc_out[1])
            emit_layer(1, hT[1], hT[2], cc_out[1],
                       wt["root2"], [wt["rel20"], wt["rel21"]], 7)

            # =============== classifier ===============
            for w in range(NW):
                ws = slice(w * WIN, (w + 1) * WIN)
                pc = ppool.tile([P, WIN], F32, tag="po", name="pc", space="PSUM")
                nc.tensor.matmul(out=pc[:], lhsT=wt["wcls"][:, 0, :],
                                 rhs=hT[2][:, ws], start=True, stop=True)
                oc = wpool.tile([P, WIN], F32, tag="oc", name="oc", bufs=1)
                nc.scalar.activation(
                    out=oc[:], in_=pc[:],
                    func=mybir.ActivationFunctionType.Identity,
                    bias=bias_t[:, 8:9])
                nc.sync.dma_start(out=outT[:, ws], in_=oc[:])

    nc.compile()
    return nc


# ---------------------------------------------------------------------------
# entry point
# ---------------------------------------------------------------------------

def kernel(**inputs):
    cfg = _derived(CFG)
    return _kernel_impl(inputs, cfg)


def _kernel_impl(inputs, cfg, trace=False):
    d = cfg
    NC, SH, SHP = d["NC"], d["SH"], d["SHP"]

    pl = build_plan(inputs["edge_index"], inputs["edge_type"], d)
    xs = prep_x(np.asarray(inputs["x"], np.float32), d)
    w = prep_weights(inputs, d)

    nc = build_bass(d, pl)

    in_maps = []
    for c in range(NC):
        m = {"xT": xs[c], "idxt": pl.idx16[c], "metat": pl.meta[c],
             "recipt": pl.recip[c], "biases": w["biases"]}
        for nm in ["wdes", "wtweet", "wnum", "wcat", "win", "root1", "rel10",
                   "rel11", "root2", "rel20", "rel21", "wcls"]:
            m[nm] = w[nm]
        in_maps.append(m)

    res = run_bass_kernel_spmd(nc, in_maps, core_ids=list(range(NC)),
                               trace=trace)

    out = np.empty((NC * SH, d["H"]), np.float32)
    for c in range(NC):
        out[c * SH:(c + 1) * SH] = res.results[c]["outT"].T[:SH]
    if trace:
        return out, res
    return out
